# revision 29
# baseline (speedup 1.0000x reference)
"""KAN expert kernel for Trainium2 (8 NeuronCores, data-parallel over batch).

Math: out[b,j] = sum_{i,g} basis_g(x[b,i]) * coeff[i,j,g] * scaling[i,j]
with cubic B-spline basis on the uniform extended grid g_m = -1 + 0.4*m.

Key identity (truncated powers): for the uniform grid, the basis is the
cardinal cubic B-spline, basis_g(x) = (1/(6h^3)) * sum_{r=0..4} w_r *
relu(x - g_{g+r})^3 with w = [1,-4,6,-4,1]. Since x in [-1,1) only
relu-features m=0..4 are nonzero, and the (linear) binomial combine is
folded into the weights on the host:
    C'[m,i,j] = (1/(6h^3)) * sum_g w_{m-g} * coeff[i,j,g] * scaling[i,j]
so each core computes Q_m = relu(x - g_m)^3 (m=0..4) and a
[512b x 2560k] @ [2560k x 512j] fp16 matmul accumulated in fp32 PSUM.

Precision: the truncated-power split cancels heavily (sum |q*W| >>
|sum q*W|), so the matmul INPUTS need >= 10 mantissa bits: fp16 passes
(7.8e-3 rel vs the 2e-2 gate) IFF the features are computed in fp32 and
rounded to fp16 exactly once:
    r_m = max(x - g_m, 0)     (DVE tensor_scalar, fp32)
    s_m = Square(x - g_m)     (ACT bias form, fp32; == r^2 wherever r>0)
    q_m = fp16(r_m * s_m)     (DVE tensor_mul, single rounding)

Schedule notes (measured on HW):
 - distinct-output tensor_tensor runs ~0.6us per [128,512] chunk but
   ~6us at [128,2048]; so muls are chunked. 2-stream ops (tensor_scalar,
   ACT activation) prefer full width. GpSimd elementwise is useless
   (35us tensor_scalar) -- it only issues the output DMAs.
 - the ACT square takes its bias from X directly, so squares never wait
   on DVE, and chunked squares for m0/m1 let the PE start early.
 - LDWEIGHTS+MATMUL pairs sustain ~216ns/MM when fed; each half may
   carry exactly one sync wait (q-producer on LDWEIGHTS, W-arrival DMA
   on MATMUL). PE declocks 2x if it idles >3.4us -> dummy warmup
   matmuls run while the DMAs land.
"""

import numpy as np

BATCH = 4096
IN_DIM = 512
OUT_DIM = 512
GRID_SIZE = 5
K = 3
N_CORES = 8
P = 128
NM = 5                      # relu^3 feature channels
BC = BATCH // N_CORES       # 512 batch rows per core
NIC = IN_DIM // P           # 4 input-dim chunks
NBC = BC // P               # 4 batch chunks (psum tiles)

_W_BINOM = np.array([1.0, -4.0, 6.0, -4.0, 1.0])

_cached = {}


def _grid_f32():
    h = 2.0 / GRID_SIZE
    return np.float32(-1.0 + h * np.arange(GRID_SIZE + 2 * K + 1))


def _build_nc(mm_dtype_name="float16", warmup=16, n_out_dma=1):
    import concourse.bass as bass
    import concourse.mybir as mybir
    from concourse.tile import TileContext
    from concourse.bass import _add_dep_helper

    dt = mybir.dt
    mm_dt = getattr(dt, mm_dtype_name)
    grid = _grid_f32()

    nc = bass.Bass()
    # bias constants for the Square activations, registered the same way
    # Bass registers its built-in consts (memset + barrier precede all
    # tile-context work, so no per-use semaphore is needed).
    for _m in range(NM):
        _v = float(-grid[_m])
        if (dt.float32, _v) not in nc.const_aps.aps:
            _t = nc.alloc_sbuf_tensor(f"const-float32-{_v}", [128, 1],
                                      dt.float32)
            nc.gpsimd.memset(_t.ap(), _v)
            nc.const_aps.aps[(dt.float32, _v)] = _t.ap()
    nc.all_engine_barrier()

    xt = nc.dram_tensor("xt", [IN_DIM, BC], dt.float32, kind="ExternalInput")
    cw = nc.dram_tensor("cw", [NM * IN_DIM, OUT_DIM], mm_dt,
                        kind="ExternalInput")
    out = nc.dram_tensor("out", [BC, OUT_DIM], mm_dt,
                         kind="ExternalOutput")

    ACTF = mybir.ActivationFunctionType
    ALU = mybir.AluOpType

    # chunk the feature chain for the first m's (early PE start); keep
    # later m's full-width (cheaper per element on 2-stream ops)
    CHUNKED = {0, 1}
    MUL_W = 1024

    with TileContext(nc) as tc:
        with tc.tile_pool(name="main", bufs=1) as pool, \
             tc.tile_pool(name="psum", bufs=1, space="PSUM") as psum_pool:
            X = pool.tile([P, NIC * BC], dt.float32, tag="X")
            CW = pool.tile([P, NM * NIC * OUT_DIM], mm_dt, tag="CW")

            # PE warmup: matmuls over a zeroed dummy tile into a spare
            # psum bank, so the PE clock is at full speed when real
            # matmuls arrive (it ramps over ~3us of continuous work).
            if warmup:
                dumb = pool.tile([P, OUT_DIM], mm_dt, tag="dumb")
                dpsum = psum_pool.tile([P, OUT_DIM], dt.float32, tag="dps",
                                       name="dps")
                nc.gpsimd.memset(dumb[:], 0.0)
                for _ in range(warmup):
                    nc.tensor.matmul(dpsum[:], dumb[:, 0:P], dumb[:],
                                     start=True, stop=True)

            # ---- input DMAs. Layout is partition-major (k = p*NIC+t) on
            # both sides of the matmul, which makes each W-group DMA 128
            # contiguous 4KB descriptors. X lands via two parallel queues
            # (sync + scalar); the later W groups are chained behind X and
            # each other with forced semaphore deps so the early tensors
            # get the full DMA bandwidth in consumption order.
            xt_r = xt.rearrange("(p t) b -> p t b", p=P)

            def dma_x(eng, t0, t1):
                return getattr(nc, eng).dma_start(
                    out=X[:, t0 * BC:t1 * BC]
                        .rearrange("p (t b) -> p t b", t=t1 - t0),
                    in_=xt_r[:, t0:t1, :])

            def dma_w(m, t0, t1):
                grp = cw[m * IN_DIM:(m + 1) * IN_DIM, :] \
                    .rearrange("(p t) j -> p t j", p=P)
                return nc.sync.dma_start(
                    out=CW[:, (m * NIC + t0) * OUT_DIM:
                           (m * NIC + t1) * OUT_DIM]
                        .rearrange("p (t j) -> p t j", t=t1 - t0),
                    in_=grp[:, t0:t1, :])

            xa = dma_x("sync", 0, 2)
            xb = dma_x("scalar", 2, NIC)
            dma_w(0, 0, 1)            # W(m0, t0): first matmuls' weights
            prev = xa
            for m, t0, t1 in [(0, 1, NIC)] + [(m, 0, NIC)
                                              for m in range(1, NM)]:
                wd = dma_w(m, t0, t1)
                _add_dep_helper(wd.ins, prev.ins, sync=True,
                                reason="stagger W DMAs behind X/previous")
                prev = wd

            def w_tile(m, ic):
                o = (m * NIC + ic) * OUT_DIM
                return CW[:, o:o + OUT_DIM]

            # ---- features
            R = [pool.tile([P, NIC * BC], dt.float32, tag=f"r{m}",
                           name=f"r{m}") for m in range(NM)]
            S = [pool.tile([P, NIC * BC], dt.float32, tag=f"s{m}",
                           name=f"s{m}") for m in range(NM)]
            Q = [pool.tile([P, NIC * BC], mm_dt, tag=f"q{m}",
                           name=f"q{m}") for m in range(NM)]

            def chunks_of(m, w=None):
                if m in CHUNKED:
                    return [slice(c * BC, (c + 1) * BC) for c in range(NIC)]
                if w:
                    return [slice(c * w, (c + 1) * w)
                            for c in range(NIC * BC // w)]
                return [slice(0, NIC * BC)]

            special_sq = {}   # inst name -> producing-DVE-covered square
            for m in [0, 1, 2, 4, 3]:
                gm = float(grid[m])
                ts_insts = []
                for sl in chunks_of(m):
                    ts_insts.append(nc.vector.tensor_scalar(
                        R[m][:, sl], X[:, sl], gm, 0.0,
                        ALU.subtract, ALU.max))
                sq_insts = []
                for sl in chunks_of(m):
                    sq_insts.append(nc.scalar.activation(
                        S[m][:, sl], X[:, sl], ACTF.Square, bias=-gm))
                if False:
                    pass
                else:
                    # q16, chunked (3-stream ops degrade at full width)
                    for sl in chunks_of(m, MUL_W):
                        nc.vector.tensor_mul(Q[m][:, sl], R[m][:, sl],
                                             S[m][:, sl])

            # ---- matmuls. m0/m1 iterate ic-outer (chunk-gated start);
            # later m's bc-outer so each psum finishes early in the m4
            # round and evictions overlap the tail.
            psums = [psum_pool.tile([P, OUT_DIM], dt.float32, tag=f"ps{b}",
                                    name=f"ps{b}")
                     for b in range(NBC)]
            O = pool.tile([P, NBC * OUT_DIM], dt.float32, tag="O")

            def mm(m, bc, ic):
                kc = m * NIC + ic
                lhsT = Q[m][:, ic * BC + bc * P: ic * BC + (bc + 1) * P]
                nc.tensor.matmul(psums[bc][:], lhsT, w_tile(m, ic),
                                 start=(kc == 0),
                                 stop=(kc == NM * NIC - 1))

            for m in range(NM):
                if m in CHUNKED:
                    for ic in range(NIC):
                        for bc in range(NBC):
                            mm(m, bc, ic)
                else:
                    for bc in range(NBC):
                        for ic in range(NIC):
                            mm(m, bc, ic)
                        if m == NM - 1:
                            nc.scalar.activation(
                                O[:, bc * OUT_DIM:(bc + 1) * OUT_DIM],
                                psums[bc][:], ACTF.Copy)

            out_dmas = []
            step = NBC // n_out_dma
            for g in range(n_out_dma):
                b0 = g * step
                od = nc.gpsimd.dma_start(
                    out=out[b0 * P:(b0 + step) * P, :]
                        .rearrange("(c p) j -> p c j", p=P),
                    in_=O[:, b0 * OUT_DIM:(b0 + step) * OUT_DIM]
                        .rearrange("p (c j) -> p c j", c=step))
                out_dmas.append(od)

    # Walrus allows one sync wait per instruction (the final drain takes
    # a few). Strip the provably redundant waits:
    #  - same-engine waits (engines are in-order FIFOs),
    #  - the q-mult's r-dependency (same-engine FIFO) and X-dependency
    #    (its ACT square already waited on X's DMA),
    #  - DMASW same-queue WAR waits on DMA copies.
    eng2sem = {"EngineType.DVE": "DVE_",
               "EngineType.Activation": "Activation_",
               "EngineType.Pool": "Pool_",
               "EngineType.PE": "PE_",
               "EngineType.SP": "SP_"}

    def _wait_val(w):
        return w.wait_value if w.wait_value is not None else -1

    bad = []
    for blk in nc.m.functions[0].blocks:
        for inst in blk.instructions:
            si = inst.sync_info
            if si is None or not si.on_wait:
                continue
            pref = eng2sem.get(str(inst.engine))
            keep = [w for w in si.on_wait
                    if pref is None
                    or not (w.ant_name or "").startswith(pref)]
            iname = type(inst).__name__
            if iname == "InstActivate" or iname == "InstActivation":
                if special_sq.get(inst.name):
                    sel = [w for w in keep
                           if (w.ant_name or "").startswith("DVE_")]
                    if sel:
                        keep = sel
            if iname == "InstTensorTensor":
                sel = [w for w in keep
                       if (w.ant_name or "").startswith("Activation_")]
                if sel:
                    keep = sel
            if iname == "InstDMACopy":
                nq = [w for w in keep
                      if not (w.ant_name or "").startswith("DMASW")]
                if nq:
                    keep = nq
            if iname == "InstDrain" and len(keep) > 1:
                out_sems = {f"DMASW{od.ins.bass_scheduled_proc - 11}_"
                            for od in out_dmas}
                sel = [w for w in keep
                       if any((w.ant_name or "").startswith(s)
                              for s in out_sems)]
                if sel:
                    keep = sel
            # same-semaphore waits collapse to the largest target value
            if len(keep) > 1:
                by_sem = {}
                for w in keep:
                    k = w.ant_name
                    if k not in by_sem or _wait_val(w) > _wait_val(by_sem[k]):
                        by_sem[k] = w
                keep = list(by_sem.values())
            if len(keep) != len(si.on_wait):
                si.on_wait = keep
            if len(keep) > 1 and iname not in ("InstDrain",):
                bad.append((inst.name, iname,
                            [w.ant_name for w in keep]))
    assert not bad, f"many-wait instructions remain: {bad}"
    return nc


def _prep_weights(spline_coeff, spline_scaling):
    # C'[m,i,j] = (1/(6h^3)) * sum_g w[m-g] * coeff[i,j,g] * scaling[i,j]
    h = 2.0 / GRID_SIZE
    c = (spline_coeff.astype(np.float64)
         * spline_scaling.astype(np.float64)[:, :, None])  # [i, j, g]
    cp = np.zeros((NM, IN_DIM, OUT_DIM), np.float64)
    for m in range(NM):
        for g in range(max(0, m - 4), m + 1):
            cp[m] += _W_BINOM[m - g] * c[:, :, g]
    cp *= 1.0 / (6.0 * h ** 3)
    return np.ascontiguousarray(
        cp.reshape(NM * IN_DIM, OUT_DIM).astype(np.float32))


def _np_mm_dtype(mm_dtype_name):
    if mm_dtype_name == "float32":
        return np.float32
    if mm_dtype_name == "float16":
        return np.float16
    if mm_dtype_name == "bfloat16":
        import ml_dtypes
        return ml_dtypes.bfloat16
    raise ValueError(mm_dtype_name)


def _run(inputs, trace=False, mm_dtype_name="float16"):
    from concourse.bass_utils import run_bass_kernel_spmd

    key = mm_dtype_name
    if key not in _cached:
        _cached[key] = _build_nc(mm_dtype_name)
    nc = _cached[key]

    x = np.asarray(inputs["x"], np.float32)
    cw = _prep_weights(np.asarray(inputs["spline_coeff"]),
                       np.asarray(inputs["spline_scaling"]))
    cw = np.ascontiguousarray(cw.astype(_np_mm_dtype(mm_dtype_name)))
    in_maps = []
    for c in range(N_CORES):
        xc = np.ascontiguousarray(x[c * BC:(c + 1) * BC, :].T)
        in_maps.append({"xt": xc, "cw": cw})
    res = run_bass_kernel_spmd(nc, in_maps, list(range(N_CORES)),
                               trace=trace)
    outp = np.concatenate([res.results[c]["out"] for c in range(N_CORES)],
                          axis=0).astype(np.float32)
    return outp, res


def kernel(**inputs):
    outp, _ = _run(inputs, trace=False)
    return outp
